# revision 58
# baseline (speedup 1.0000x reference)
"""Trainium2 Bass kernel for nn_Attention_81716047774180.

Dense transformer attention block:
  qkv = x @ qkv_w + qkv_b ; split into q,k,v heads [B,H,N,d]
  attn = softmax(q k^T * scale + rel_pos_bias) ; out = (attn @ v) @ proj_w + proj_b

Distribution: pure data-parallel over batch B=64 across 8 NeuronCores
(8 batches per core, no collectives).

Device algorithm (per core, bf16 compute, fp32 accumulation):
  - host folds: scale into q-weights, rel-index gather + exp + transpose into
    an expB table, x transposed to [DIM, tokens] so no on-device transposes.
  - qk^T part computed transposed ([outdim, token]) so q^T,k^T land in [d, N]
    layout; v computed direct ([token, outdim]) with a ones column appended
    per head (gives softmax denominators for free from the P@V matmul).
  - S^T = k q^T per head ([nk, nq]) -> exp -> * expB -> P^T, so P@V needs no
    transposes: out^T[d, nq] = v^T P^T via lhsT=v.  S^T matmuls of even/odd
    heads alternate PE row groups (tile_position auto) so they co-execute.
  - denominators of all 12 heads batched into one [12, N] reciprocal
    (via base-64 staging row + tiny partition-shift DMAs), broadcast over
    partitions with indicator-matrix matmuls, normalization fused into an
    in-place multiply on attn^T.
  - software pipeline over batches: S^T burst (PE) -> next batch's
    projections (PE) overlap the exp/mul wavefront (ACT/DVE) -> P@V ->
    normalize -> proj.
"""

import sys

import numpy as np
import ml_dtypes

if "/opt/trn_rl_repo" not in sys.path:
    sys.path.insert(0, "/opt/trn_rl_repo")

B, N, DIM, H, d = 64, 320, 768, 12, 64
N_CORES = 8
B_LOC = B // N_CORES
QK = 2 * DIM          # 1536 (q^T and k^T rows)
NKC = [128, 128, 64]  # nk chunking of N=320
MT = [128, 128, 64]   # token chunking of N=320
VW = 65               # v columns per head incl. ones column

_BF16 = ml_dtypes.bfloat16


def build_nc(n_batches=B_LOC, has_qkb=False, has_vb=False, has_pb=False):
    import concourse.bass as bass  # noqa: F401
    import concourse.tile as tile
    from concourse import bacc, mybir
    from contextlib import ExitStack

    bf16 = mybir.dt.bfloat16
    f32 = mybir.dt.float32
    ACT = mybir.ActivationFunctionType

    nc = bacc.Bacc("TRN2", target_bir_lowering=False, debug=False,
                   num_devices=N_CORES)

    toks = n_batches * N
    ind_d = nc.dram_tensor("ind", [H, DIM], bf16, kind="ExternalInput").ap()
    xT_d = nc.dram_tensor("xT", [DIM, toks], bf16, kind="ExternalInput").ap()
    wqk_d = nc.dram_tensor("w_qk", [DIM, QK], bf16, kind="ExternalInput").ap()
    wv_d = nc.dram_tensor("w_v", [DIM, DIM], bf16, kind="ExternalInput").ap()
    pw_d = nc.dram_tensor("proj_w", [DIM, DIM], bf16, kind="ExternalInput").ap()
    eB_d = nc.dram_tensor("expB", [H, N, N], bf16, kind="ExternalInput").ap()
    if has_qkb:
        qkb_d = nc.dram_tensor("qkb", [12, 128], f32, kind="ExternalInput").ap()
    if has_vb:
        vb_d = nc.dram_tensor("vb", [1, DIM], bf16, kind="ExternalInput").ap()
    if has_pb:
        pb_d = nc.dram_tensor("pb", [1, DIM], bf16, kind="ExternalInput").ap()
    out_d = nc.dram_tensor("out", [toks, DIM], f32, kind="ExternalOutput").ap()

    with tile.TileContext(nc) as tc, ExitStack() as ctx:
        sing = ctx.enter_context(tc.tile_pool(name="sing", bufs=1))
        qkT_p = ctx.enter_context(tc.tile_pool(name="qkT", bufs=2))
        v_p = ctx.enter_context(tc.tile_pool(name="v", bufs=3))
        pe_p = ctx.enter_context(tc.tile_pool(name="pe", bufs=4))
        pt_p = ctx.enter_context(tc.tile_pool(name="pt", bufs=76))
        rc_p = ctx.enter_context(tc.tile_pool(name="rc", bufs=2))
        aT_p = ctx.enter_context(tc.tile_pool(name="aT", bufs=2))
        o_p = ctx.enter_context(tc.tile_pool(name="o", bufs=2))
        # Separate PSUM pools so projection matmuls are not slot-blocked
        # behind attention tiles waiting on ACT exps (8 banks total).
        ps_s = ctx.enter_context(tc.tile_pool(name="ps_s", bufs=3, space="PSUM"))
        ps_o = ctx.enter_context(tc.tile_pool(name="ps_o", bufs=3, space="PSUM"))
        ps_g = ctx.enter_context(tc.tile_pool(name="ps_g", bufs=2, space="PSUM"))

        # ---- resident constants ----
        # DMA order matters for the ramp: xT + qkv weights feed the first
        # projections, so they go first; expB is not read until the first
        # S^T results exp (~20us in), proj weights not until later still.
        wqk_t = []
        wv_t = []
        pw_t = []
        xT_t = []
        for kc in range(6):
            t = sing.tile([128, toks], bf16, tag=f"xT{kc}", name=f"xT{kc}")
            nc.sync.dma_start(t[:], xT_d[kc * 128:(kc + 1) * 128, :])
            xT_t.append(t)
            t = sing.tile([128, QK], bf16, tag=f"wqk{kc}", name=f"wqk{kc}")
            nc.sync.dma_start(t[:], wqk_d[kc * 128:(kc + 1) * 128, :])
            wqk_t.append(t)
        for kc in range(6):
            t = sing.tile([128, DIM], bf16, tag=f"wv{kc}", name=f"wv{kc}")
            nc.sync.dma_start(t[:], wv_d[kc * 128:(kc + 1) * 128, :])
            wv_t.append(t)
        eB_t = [[None] * 3 for _ in range(H)]
        for h in range(H):
            for c in range(3):
                ck = NKC[c]
                t = sing.tile([128, N], bf16, tag=f"eB{h}_{c}",
                              name=f"eB{h}_{c}")
                nc.sync.dma_start(t[:ck, :], eB_d[h, c * 128:c * 128 + ck, :])
                eB_t[h][c] = t
        for kc in range(6):
            t = sing.tile([128, DIM], bf16, tag=f"pw{kc}", name=f"pw{kc}")
            nc.sync.dma_start(t[:], pw_d[kc * 128:(kc + 1) * 128, :])
            pw_t.append(t)
        ind_t = sing.tile([128, DIM], bf16, tag="ind")
        nc.sync.dma_start(ind_t[:H, :], ind_d[:, :])
        if has_qkb:
            qkb_t = sing.tile([128, 12], f32, tag="qkb")
            nc.sync.dma_start(qkb_t[:], qkb_d.rearrange("t p -> p t"))
        if has_vb or has_pb:
            ones_t = sing.tile([128, 128], bf16, tag="ones")
            nc.vector.memset(ones_t[:], 1.0)
        if has_vb:
            vb_t = sing.tile([1, DIM], bf16, tag="vb")
            nc.sync.dma_start(vb_t[:], vb_d[:, :])
        if has_pb:
            pb_t = sing.tile([1, DIM], bf16, tag="pb")
            nc.sync.dma_start(pb_t[:], pb_d[:, :])

        def qkv_units(b, boost=False):
            """qk^T + v projection units for batch b -> (units, state).
            boost=True (prologue only): round-robin all three PSUM pools,
            which are otherwise idle before the pipeline fills."""
            t0 = b * N
            qkT = [qkT_p.tile([128, N], bf16, tag=f"qkT{mt}", name=f"qkT{mt}")
                   for mt in range(12)]
            vt = [v_p.tile([128, H * VW], bf16, tag=f"v{mt}", name=f"v{mt}")
                  for mt in range(3)]
            pools = [ps_g, ps_s, ps_o] if boost else [ps_g]
            tags = ["psg", "pss", "pso"] if boost else ["psg"]
            pidx = [0]

            def pick():
                p, tg = pools[pidx[0] % len(pools)], tags[pidx[0] % len(tags)]
                pidx[0] += 1
                return p.tile([128, 384], f32, tag=tg, name=tg)

            def qkT_unit(mt):
                ps = pick()
                for kc in range(6):
                    nc.tensor.matmul(
                        ps[:, :N],
                        lhsT=wqk_t[kc][:, mt * 128:(mt + 1) * 128],
                        rhs=xT_t[kc][:, t0:t0 + N],
                        start=(kc == 0), stop=(kc == 5))
                if has_qkb:
                    nc.scalar.activation(qkT[mt][:], ps[:, :N], ACT.Copy,
                                         bias=qkb_t[:, mt:mt + 1])
                else:
                    nc.vector.tensor_copy(qkT[mt][:], ps[:, :N])

            def v_unit(mt, nh):
                rows = MT[mt]
                t = vt[mt]
                ps = pick()
                for kc in range(6):
                    nc.tensor.matmul(
                        ps[:rows, :384],
                        lhsT=xT_t[kc][:, t0 + mt * 128:t0 + mt * 128 + rows],
                        rhs=wv_t[kc][:, nh * 384:(nh + 1) * 384],
                        start=(kc == 0), stop=(kc == 5 and not has_vb))
                if has_vb:
                    nc.tensor.matmul(
                        ps[:rows, :384],
                        lhsT=ones_t[0:1, 0:rows],
                        rhs=vb_t[0:1, nh * 384:(nh + 1) * 384],
                        start=False, stop=True)
                nc.scalar.activation(
                    t.rearrange("p (h c) -> p h c", c=VW)[:rows, nh * 6:(nh + 1) * 6, 0:64],
                    ps.rearrange("p (h c) -> p h c", c=64)[:rows, 0:6, :],
                    ACT.Copy)
                if nh == 1:
                    nc.vector.memset(
                        t.rearrange("p (h c) -> p h c", c=VW)[:rows, :, 64:65],
                        1.0)

            units = [lambda mt=mt: qkT_unit(mt) for mt in range(12)]
            units += [lambda mt=mt, nh=nh: v_unit(mt, nh)
                      for mt in range(3) for nh in range(2)]
            return units, (qkT, vt)

        def score_units(b, state, pTs):
            """18 units, one per (pair, chunk): S^T of even+odd head (in
            different PE row groups, so they co-execute) + exp + bias-mul.
            The c==0 bias-mul runs on the otherwise idle GpSimd engine."""
            qkT, vt = state

            def unit(j, c):
                ck = NKC[c]
                for r in range(2):
                    h = 2 * j + r
                    rb = r * 64
                    ps = ps_s.tile([128, N], f32, tag="pss", name="pss")
                    nc.tensor.matmul(
                        ps[:ck, :N],
                        lhsT=qkT[6 + j][rb:rb + 64, c * 128:c * 128 + ck],
                        rhs=qkT[j][rb:rb + 64, 0:N],
                        start=True, stop=True)
                    pexp = pe_p.tile([128, N], bf16, tag="pexp", name="pexp")
                    nc.scalar.activation(pexp[:ck, :], ps[:ck, :N], ACT.Exp)
                    pT = pt_p.tile([128, N], bf16, tag="pT", name="pT")
                    eng = nc.gpsimd if c <= 1 else nc.vector
                    eng.tensor_mul(pT[:ck, :], pexp[:ck, :], eB_t[h][c][:ck, :])
                    pTs[h][c] = pT

            return [lambda j=j, c=c: unit(j, c)
                    for j in range(6) for c in range(3)]

        def av_units(b, state, pTs, attnT, dens, dstage, boost=False):
            """12 units: P@V accumulation + denom extraction + unnorm evict.
            boost=True (last batch): also borrow the idle scores PSUM pool."""
            qkT, vt = state

            def unit(h):
                j, r = divmod(h, 2)
                rb = r * 64
                if boost and h % 2 == 1:
                    po = ps_s.tile([128, N], f32, tag="pss", name="pss")
                else:
                    po = ps_o.tile([128, N], f32, tag="pso", name="pso")
                for c in range(3):
                    ck = NKC[c]
                    nc.tensor.matmul(
                        po[0:VW, :N],
                        lhsT=vt[c][0:ck, h * VW:(h + 1) * VW],
                        rhs=pTs[h][c][0:ck, :],
                        start=(c == 0), stop=(c == 2))
                # denom row 64 -> a 32-aligned staging slot (engines cannot
                # write non-32-aligned partitions); once a slot group of 4
                # heads is staged, one partition-strided DMA packs them into
                # rows 4f..4f+3 of `dens`.
                sr, sc_ = 32 * (h % 4), (h // 4) * N
                with nc.allow_low_precision(reason="softmax denom in bf16"):
                    nc.vector.tensor_copy(dstage[sr:sr + 1, sc_:sc_ + N],
                                          po[64:65, :N])
                if h % 4 == 3:
                    f = h // 4
                    nc.sync.dma_start(
                        out=dens[4 * f:4 * f + 4, :],
                        in_=dstage[0:128:32, f * N:(f + 1) * N])
                nc.scalar.activation(attnT[j][rb:rb + 64, :], po[0:64, :N],
                                     ACT.Copy)

            return [lambda h=h: unit(h) for h in range(H)]

        def norm_proj_units(b, attnT, dens, boost=False):
            """Batched reciprocal, per-pair normalize, projection + out."""
            t0 = b * N
            den_r = rc_p.tile([128, N], bf16, tag="den_r", name="den_r")

            def recip_unit():
                with nc.allow_low_precision(reason="softmax denom recip bf16"):
                    nc.vector.reciprocal(den_r[:H, :], dens[:H, :])

            def norm_unit(j):
                ps_b = ps_o.tile([128, N], f32, tag="pso", name="pso")
                nc.tensor.matmul(
                    ps_b[:, :N],
                    lhsT=ind_t[0:H, j * 128:(j + 1) * 128],
                    rhs=den_r[0:H, :],
                    start=True, stop=True)
                nc.vector.tensor_mul(attnT[j][:], attnT[j][:], ps_b[:, :N])

            def proj_unit(mt):
                rows = MT[mt]
                o_t = o_p.tile([128, DIM], f32, tag="o", name="o")
                for nh in range(2):
                    if boost and nh == 1:
                        ps = ps_s.tile([128, 384], f32, tag="pss", name="pss")
                    else:
                        ps = ps_g.tile([128, 384], f32, tag="psg", name="psg")
                    for j in range(6):
                        nc.tensor.matmul(
                            ps[:rows, :384],
                            lhsT=attnT[j][:, mt * 128:mt * 128 + rows],
                            rhs=pw_t[j][:, nh * 384:(nh + 1) * 384],
                            start=(j == 0), stop=(j == 5 and not has_pb))
                    if has_pb:
                        nc.tensor.matmul(
                            ps[:rows, :384],
                            lhsT=ones_t[0:1, 0:rows],
                            rhs=pb_t[0:1, nh * 384:(nh + 1) * 384],
                            start=False, stop=True)
                    nc.vector.tensor_copy(
                        o_t[:rows, nh * 384:(nh + 1) * 384], ps[:rows, :384])
                nc.sync.dma_start(
                    out_d[t0 + mt * 128:t0 + mt * 128 + rows, :], o_t[:rows, :])

            return ([recip_unit]
                    + [lambda j=j: norm_unit(j) for j in range(6)]
                    + [lambda mt=mt: proj_unit(mt) for mt in range(3)])

        # Three-deep software pipeline over batches. Each step interleaves:
        #   - batch b's S^T/exp/bias-mul units   (PE + ACT/DVE wavefront)
        #   - batch b+1's qkT/v projection units (dense PE, independent)
        #   - batch b-1's P@V / normalize / proj (inputs all ready -> these
        #     fill every stall the exp wavefront would otherwise cause)
        # The three streams use disjoint PSUM pools (3+3+2 banks).
        qv_units, state = qkv_units(0, boost=True)
        for u in qv_units:
            u()
        tail = []          # av/norm/proj units of batch b-1
        prev_ctx = None
        for b in range(n_batches):
            attnT = [aT_p.tile([128, N], bf16, tag=f"aT{j}", name=f"aT{j}")
                     for j in range(6)]
            dens = rc_p.tile([128, N], bf16, tag="dens", name="dens")
            dstage = rc_p.tile([128, 3 * N], bf16, tag="dstage", name="dstage")
            pTs = [[None] * 3 for _ in range(H)]
            sc = score_units(b, state, pTs)
            if b + 1 < n_batches:
                qv, nstate = qkv_units(b + 1)
            else:
                qv, nstate = [], None
            ns, nq, nt = len(sc), len(qv), len(tail)
            for i in range(max(ns, nq, nt)):
                if i < nt:
                    tail[i]()
                if i < ns:
                    sc[i]()
                if i < nq:
                    qv[i]()
            last = b == n_batches - 1
            tail = (av_units(b, state, pTs, attnT, dens, dstage, boost=last)
                    + norm_proj_units(b, attnT, dens, boost=last))
            state = nstate
        for u in tail:
            u()

    nc.compile()
    return nc


def prep_host(x, qkv_w, qkv_b, proj_w, proj_b, rpb_table, rel_index):
    """Host-side preprocessing: fold scale/gather/exp/transposes."""
    scale = d ** -0.5
    qkv_w = np.asarray(qkv_w, np.float32)
    w_qk = np.concatenate(
        [qkv_w[:, :DIM] * scale, qkv_w[:, DIM:QK]], axis=1).astype(_BF16)
    w_v = np.ascontiguousarray(qkv_w[:, QK:]).astype(_BF16)
    pw = np.asarray(proj_w, np.float32).astype(_BF16)
    bias = np.asarray(rpb_table)[:, np.asarray(rel_index)]       # [H, nq, nk]
    expB = np.exp(bias.transpose(0, 2, 1)).astype(_BF16)          # [H, nk, nq]
    expB = np.ascontiguousarray(expB)
    qkv_b = np.asarray(qkv_b, np.float32)
    qkb = np.concatenate([qkv_b[:DIM] * scale, qkv_b[DIM:QK]])
    vb = qkv_b[QK:]
    has_qkb = bool(np.any(qkb))
    has_vb = bool(np.any(vb))
    has_pb = bool(np.any(np.asarray(proj_b)))

    ind = np.zeros((H, DIM), dtype=_BF16)
    for h in range(H):
        ind[h, h * 64:(h + 1) * 64] = 1.0
    shared = {"w_qk": w_qk, "w_v": w_v, "proj_w": pw, "expB": expB, "ind": ind}
    if has_qkb:
        shared["qkb"] = np.ascontiguousarray(qkb.reshape(12, 128)).astype(np.float32)
    if has_vb:
        shared["vb"] = vb.reshape(1, DIM).astype(_BF16)
    if has_pb:
        shared["pb"] = np.asarray(proj_b).reshape(1, DIM).astype(_BF16)

    in_maps = []
    for c in range(N_CORES):
        xs = np.asarray(x[c * B_LOC:(c + 1) * B_LOC], np.float32)
        xT = np.ascontiguousarray(xs.reshape(B_LOC * N, DIM).T).astype(_BF16)
        m = {"xT": xT}
        m.update(shared)
        in_maps.append(m)
    return in_maps, has_qkb, has_vb, has_pb


_NC_CACHE = {}


def kernel(x, qkv_w, qkv_b, proj_w, proj_b, rpb_table, rel_index):
    from concourse.bass_utils import run_bass_kernel_spmd

    in_maps, has_qkb, has_vb, has_pb = prep_host(
        x, qkv_w, qkv_b, proj_w, proj_b, rpb_table, rel_index)
    key = (has_qkb, has_vb, has_pb)
    if key not in _NC_CACHE:
        _NC_CACHE[key] = build_nc(B_LOC, has_qkb, has_vb, has_pb)
    nc = _NC_CACHE[key]
    res = run_bass_kernel_spmd(nc, in_maps, core_ids=list(range(N_CORES)))
    out = np.concatenate(
        [res.results[c]["out"].reshape(B_LOC, N, DIM) for c in range(N_CORES)],
        axis=0)
    return out.astype(np.float32)


# revision 59
# speedup vs baseline: 1.0079x; 1.0079x over previous
"""Trainium2 Bass kernel for nn_Attention_81716047774180.

Dense transformer attention block:
  qkv = x @ qkv_w + qkv_b ; split into q,k,v heads [B,H,N,d]
  attn = softmax(q k^T * scale + rel_pos_bias) ; out = (attn @ v) @ proj_w + proj_b

Distribution: pure data-parallel over batch B=64 across 8 NeuronCores
(8 batches per core, no collectives).

Device algorithm (per core, bf16 compute, fp32 accumulation):
  - host folds: scale into q-weights, rel-index gather + exp + transpose into
    an expB table, x transposed to [DIM, tokens] so no on-device transposes.
  - qk^T part computed transposed ([outdim, token]) so q^T,k^T land in [d, N]
    layout; v computed direct ([token, outdim]) with a ones column appended
    per head (gives softmax denominators for free from the P@V matmul).
  - S^T = k q^T per head ([nk, nq]) -> exp -> * expB -> P^T, so P@V needs no
    transposes: out^T[d, nq] = v^T P^T via lhsT=v.  S^T matmuls of even/odd
    heads alternate PE row groups (tile_position auto) so they co-execute.
  - denominators of all 12 heads batched into one [12, N] reciprocal
    (via base-64 staging row + tiny partition-shift DMAs), broadcast over
    partitions with indicator-matrix matmuls, normalization fused into an
    in-place multiply on attn^T.
  - software pipeline over batches: S^T burst (PE) -> next batch's
    projections (PE) overlap the exp/mul wavefront (ACT/DVE) -> P@V ->
    normalize -> proj.
"""

import sys

import numpy as np
import ml_dtypes

if "/opt/trn_rl_repo" not in sys.path:
    sys.path.insert(0, "/opt/trn_rl_repo")

B, N, DIM, H, d = 64, 320, 768, 12, 64
N_CORES = 8
B_LOC = B // N_CORES
QK = 2 * DIM          # 1536 (q^T and k^T rows)
NKC = [128, 128, 64]  # nk chunking of N=320
MT = [128, 128, 64]   # token chunking of N=320
VW = 65               # v columns per head incl. ones column

_BF16 = ml_dtypes.bfloat16


def build_nc(n_batches=B_LOC, has_qkb=False, has_vb=False, has_pb=False):
    import concourse.bass as bass  # noqa: F401
    import concourse.tile as tile
    from concourse import bacc, mybir
    from contextlib import ExitStack

    bf16 = mybir.dt.bfloat16
    f32 = mybir.dt.float32
    ACT = mybir.ActivationFunctionType

    nc = bacc.Bacc("TRN2", target_bir_lowering=False, debug=False,
                   num_devices=N_CORES)

    toks = n_batches * N
    ind_d = nc.dram_tensor("ind", [H, DIM], bf16, kind="ExternalInput").ap()
    xT_d = nc.dram_tensor("xT", [DIM, toks], bf16, kind="ExternalInput").ap()
    wqk_d = nc.dram_tensor("w_qk", [DIM, QK], bf16, kind="ExternalInput").ap()
    wv_d = nc.dram_tensor("w_v", [DIM, DIM], bf16, kind="ExternalInput").ap()
    pw_d = nc.dram_tensor("proj_w", [DIM, DIM], bf16, kind="ExternalInput").ap()
    eB_d = nc.dram_tensor("expB", [H, N, N], bf16, kind="ExternalInput").ap()
    if has_qkb:
        qkb_d = nc.dram_tensor("qkb", [12, 128], f32, kind="ExternalInput").ap()
    if has_vb:
        vb_d = nc.dram_tensor("vb", [1, DIM], bf16, kind="ExternalInput").ap()
    if has_pb:
        pb_d = nc.dram_tensor("pb", [1, DIM], bf16, kind="ExternalInput").ap()
    out_d = nc.dram_tensor("out", [toks, DIM], f32, kind="ExternalOutput").ap()

    with tile.TileContext(nc) as tc, ExitStack() as ctx:
        sing = ctx.enter_context(tc.tile_pool(name="sing", bufs=1))
        qkT_p = ctx.enter_context(tc.tile_pool(name="qkT", bufs=2))
        v_p = ctx.enter_context(tc.tile_pool(name="v", bufs=3))
        pe_p = ctx.enter_context(tc.tile_pool(name="pe", bufs=4))
        pt_p = ctx.enter_context(tc.tile_pool(name="pt", bufs=76))
        rc_p = ctx.enter_context(tc.tile_pool(name="rc", bufs=2))
        aT_p = ctx.enter_context(tc.tile_pool(name="aT", bufs=3))
        o_p = ctx.enter_context(tc.tile_pool(name="o", bufs=2))
        # Separate PSUM pools so projection matmuls are not slot-blocked
        # behind attention tiles waiting on ACT exps (8 banks total).
        ps_s = ctx.enter_context(tc.tile_pool(name="ps_s", bufs=3, space="PSUM"))
        ps_o = ctx.enter_context(tc.tile_pool(name="ps_o", bufs=3, space="PSUM"))
        ps_g = ctx.enter_context(tc.tile_pool(name="ps_g", bufs=2, space="PSUM"))

        # ---- resident constants ----
        # DMA order matters for the ramp: xT + qkv weights feed the first
        # projections, so they go first; expB is not read until the first
        # S^T results exp (~20us in), proj weights not until later still.
        wqk_t = []
        wv_t = []
        pw_t = []
        xT_t = []
        for kc in range(6):
            t = sing.tile([128, toks], bf16, tag=f"xT{kc}", name=f"xT{kc}")
            nc.sync.dma_start(t[:], xT_d[kc * 128:(kc + 1) * 128, :])
            xT_t.append(t)
            t = sing.tile([128, QK], bf16, tag=f"wqk{kc}", name=f"wqk{kc}")
            nc.sync.dma_start(t[:], wqk_d[kc * 128:(kc + 1) * 128, :])
            wqk_t.append(t)
        for kc in range(6):
            t = sing.tile([128, DIM], bf16, tag=f"wv{kc}", name=f"wv{kc}")
            nc.sync.dma_start(t[:], wv_d[kc * 128:(kc + 1) * 128, :])
            wv_t.append(t)
        eB_t = [[None] * 3 for _ in range(H)]
        for h in range(H):
            for c in range(3):
                ck = NKC[c]
                t = sing.tile([128, N], bf16, tag=f"eB{h}_{c}",
                              name=f"eB{h}_{c}")
                nc.sync.dma_start(t[:ck, :], eB_d[h, c * 128:c * 128 + ck, :])
                eB_t[h][c] = t
        for kc in range(6):
            t = sing.tile([128, DIM], bf16, tag=f"pw{kc}", name=f"pw{kc}")
            nc.sync.dma_start(t[:], pw_d[kc * 128:(kc + 1) * 128, :])
            pw_t.append(t)
        ind_t = sing.tile([128, DIM], bf16, tag="ind")
        nc.sync.dma_start(ind_t[:H, :], ind_d[:, :])
        if has_qkb:
            qkb_t = sing.tile([128, 12], f32, tag="qkb")
            nc.sync.dma_start(qkb_t[:], qkb_d.rearrange("t p -> p t"))
        if has_vb or has_pb:
            ones_t = sing.tile([128, 128], bf16, tag="ones")
            nc.vector.memset(ones_t[:], 1.0)
        if has_vb:
            vb_t = sing.tile([1, DIM], bf16, tag="vb")
            nc.sync.dma_start(vb_t[:], vb_d[:, :])
        if has_pb:
            pb_t = sing.tile([1, DIM], bf16, tag="pb")
            nc.sync.dma_start(pb_t[:], pb_d[:, :])

        def qkv_units(b, boost=False):
            """qk^T + v projection units for batch b -> (units, state).
            boost=True (prologue only): round-robin all three PSUM pools,
            which are otherwise idle before the pipeline fills."""
            t0 = b * N
            qkT = [qkT_p.tile([128, N], bf16, tag=f"qkT{mt}", name=f"qkT{mt}")
                   for mt in range(12)]
            vt = [v_p.tile([128, H * VW], bf16, tag=f"v{mt}", name=f"v{mt}")
                  for mt in range(3)]
            pools = [ps_g, ps_s, ps_o] if boost else [ps_g]
            tags = ["psg", "pss", "pso"] if boost else ["psg"]
            pidx = [0]

            def pick():
                p, tg = pools[pidx[0] % len(pools)], tags[pidx[0] % len(tags)]
                pidx[0] += 1
                return p.tile([128, 384], f32, tag=tg, name=tg)

            def qkT_unit(mt):
                ps = pick()
                for kc in range(6):
                    nc.tensor.matmul(
                        ps[:, :N],
                        lhsT=wqk_t[kc][:, mt * 128:(mt + 1) * 128],
                        rhs=xT_t[kc][:, t0:t0 + N],
                        start=(kc == 0), stop=(kc == 5))
                if has_qkb:
                    nc.scalar.activation(qkT[mt][:], ps[:, :N], ACT.Copy,
                                         bias=qkb_t[:, mt:mt + 1])
                else:
                    nc.vector.tensor_copy(qkT[mt][:], ps[:, :N])

            def v_unit(mt, nh):
                rows = MT[mt]
                t = vt[mt]
                ps = pick()
                for kc in range(6):
                    nc.tensor.matmul(
                        ps[:rows, :384],
                        lhsT=xT_t[kc][:, t0 + mt * 128:t0 + mt * 128 + rows],
                        rhs=wv_t[kc][:, nh * 384:(nh + 1) * 384],
                        start=(kc == 0), stop=(kc == 5 and not has_vb))
                if has_vb:
                    nc.tensor.matmul(
                        ps[:rows, :384],
                        lhsT=ones_t[0:1, 0:rows],
                        rhs=vb_t[0:1, nh * 384:(nh + 1) * 384],
                        start=False, stop=True)
                nc.scalar.activation(
                    t.rearrange("p (h c) -> p h c", c=VW)[:rows, nh * 6:(nh + 1) * 6, 0:64],
                    ps.rearrange("p (h c) -> p h c", c=64)[:rows, 0:6, :],
                    ACT.Copy)
                if nh == 1:
                    nc.vector.memset(
                        t.rearrange("p (h c) -> p h c", c=VW)[:rows, :, 64:65],
                        1.0)

            units = [lambda mt=mt: qkT_unit(mt) for mt in range(12)]
            units += [lambda mt=mt, nh=nh: v_unit(mt, nh)
                      for mt in range(3) for nh in range(2)]
            return units, (qkT, vt)

        def score_units(b, state, pTs):
            """18 units, one per (pair, chunk): S^T of even+odd head (in
            different PE row groups, so they co-execute) + exp + bias-mul.
            The c==0 bias-mul runs on the otherwise idle GpSimd engine."""
            qkT, vt = state

            def unit(j, c):
                ck = NKC[c]
                for r in range(2):
                    h = 2 * j + r
                    rb = r * 64
                    ps = ps_s.tile([128, N], f32, tag="pss", name="pss")
                    nc.tensor.matmul(
                        ps[:ck, :N],
                        lhsT=qkT[6 + j][rb:rb + 64, c * 128:c * 128 + ck],
                        rhs=qkT[j][rb:rb + 64, 0:N],
                        start=True, stop=True)
                    pexp = pe_p.tile([128, N], bf16, tag="pexp", name="pexp")
                    nc.scalar.activation(pexp[:ck, :], ps[:ck, :N], ACT.Exp)
                    pT = pt_p.tile([128, N], bf16, tag="pT", name="pT")
                    eng = nc.gpsimd if c <= 1 else nc.vector
                    eng.tensor_mul(pT[:ck, :], pexp[:ck, :], eB_t[h][c][:ck, :])
                    pTs[h][c] = pT

            return [lambda j=j, c=c: unit(j, c)
                    for j in range(6) for c in range(3)]

        def av_units(b, state, pTs, attnT, dens, dstage, boost=False):
            """12 units: P@V accumulation + denom extraction + unnorm evict.
            boost=True (last batch): also borrow the idle scores PSUM pool."""
            qkT, vt = state

            def unit(h):
                j, r = divmod(h, 2)
                rb = r * 64
                if boost and h % 2 == 1:
                    po = ps_s.tile([128, N], f32, tag="pss", name="pss")
                else:
                    po = ps_o.tile([128, N], f32, tag="pso", name="pso")
                for c in range(3):
                    ck = NKC[c]
                    nc.tensor.matmul(
                        po[0:VW, :N],
                        lhsT=vt[c][0:ck, h * VW:(h + 1) * VW],
                        rhs=pTs[h][c][0:ck, :],
                        start=(c == 0), stop=(c == 2))
                # denom row 64 -> a 32-aligned staging slot (engines cannot
                # write non-32-aligned partitions); once a slot group of 4
                # heads is staged, one partition-strided DMA packs them into
                # rows 4f..4f+3 of `dens`.
                sr, sc_ = 32 * (h % 4), (h // 4) * N
                with nc.allow_low_precision(reason="softmax denom in bf16"):
                    nc.vector.tensor_copy(dstage[sr:sr + 1, sc_:sc_ + N],
                                          po[64:65, :N])
                if h % 4 == 3:
                    f = h // 4
                    nc.sync.dma_start(
                        out=dens[4 * f:4 * f + 4, :],
                        in_=dstage[0:128:32, f * N:(f + 1) * N])
                nc.scalar.activation(attnT[j][rb:rb + 64, :], po[0:64, :N],
                                     ACT.Copy)

            return [lambda h=h: unit(h) for h in range(H)]

        def norm_proj_units(b, attnT, dens, boost=False):
            """Batched reciprocal, per-pair normalize, projection + out."""
            t0 = b * N
            den_r = rc_p.tile([128, N], bf16, tag="den_r", name="den_r")

            def recip_unit():
                with nc.allow_low_precision(reason="softmax denom recip bf16"):
                    nc.vector.reciprocal(den_r[:H, :], dens[:H, :])

            def norm_unit(j):
                ps_b = ps_o.tile([128, N], f32, tag="pso", name="pso")
                nc.tensor.matmul(
                    ps_b[:, :N],
                    lhsT=ind_t[0:H, j * 128:(j + 1) * 128],
                    rhs=den_r[0:H, :],
                    start=True, stop=True)
                nc.vector.tensor_mul(attnT[j][:], attnT[j][:], ps_b[:, :N])

            def proj_unit(mt):
                rows = MT[mt]
                o_t = o_p.tile([128, DIM], f32, tag="o", name="o")
                for nh in range(2):
                    if boost and nh == 1:
                        ps = ps_s.tile([128, 384], f32, tag="pss", name="pss")
                    else:
                        ps = ps_g.tile([128, 384], f32, tag="psg", name="psg")
                    for j in range(6):
                        nc.tensor.matmul(
                            ps[:rows, :384],
                            lhsT=attnT[j][:, mt * 128:mt * 128 + rows],
                            rhs=pw_t[j][:, nh * 384:(nh + 1) * 384],
                            start=(j == 0), stop=(j == 5 and not has_pb))
                    if has_pb:
                        nc.tensor.matmul(
                            ps[:rows, :384],
                            lhsT=ones_t[0:1, 0:rows],
                            rhs=pb_t[0:1, nh * 384:(nh + 1) * 384],
                            start=False, stop=True)
                    nc.vector.tensor_copy(
                        o_t[:rows, nh * 384:(nh + 1) * 384], ps[:rows, :384])
                nc.sync.dma_start(
                    out_d[t0 + mt * 128:t0 + mt * 128 + rows, :], o_t[:rows, :])

            return ([recip_unit]
                    + [lambda j=j: norm_unit(j) for j in range(6)]
                    + [lambda mt=mt: proj_unit(mt) for mt in range(3)])

        # Three-deep software pipeline over batches. Each step interleaves:
        #   - batch b's S^T/exp/bias-mul units   (PE + ACT/DVE wavefront)
        #   - batch b+1's qkT/v projection units (dense PE, independent)
        #   - batch b-1's P@V / normalize / proj (inputs all ready -> these
        #     fill every stall the exp wavefront would otherwise cause)
        # The three streams use disjoint PSUM pools (3+3+2 banks).
        qv_units, state = qkv_units(0, boost=True)
        for u in qv_units:
            u()
        tail = []          # av/norm/proj units of batch b-1
        prev_ctx = None
        for b in range(n_batches):
            attnT = [aT_p.tile([128, N], bf16, tag=f"aT{j}", name=f"aT{j}")
                     for j in range(6)]
            dens = rc_p.tile([128, N], bf16, tag="dens", name="dens")
            dstage = rc_p.tile([128, 3 * N], bf16, tag="dstage", name="dstage")
            pTs = [[None] * 3 for _ in range(H)]
            sc = score_units(b, state, pTs)
            if b + 1 < n_batches:
                qv, nstate = qkv_units(b + 1)
            else:
                qv, nstate = [], None
            ns, nq, nt = len(sc), len(qv), len(tail)
            for i in range(max(ns, nq, nt)):
                if i < nt:
                    tail[i]()
                if i < ns:
                    sc[i]()
                if i < nq:
                    qv[i]()
            last = b == n_batches - 1
            tail = (av_units(b, state, pTs, attnT, dens, dstage, boost=last)
                    + norm_proj_units(b, attnT, dens, boost=last))
            state = nstate
        for u in tail:
            u()

    nc.compile()
    return nc


def prep_host(x, qkv_w, qkv_b, proj_w, proj_b, rpb_table, rel_index):
    """Host-side preprocessing: fold scale/gather/exp/transposes."""
    scale = d ** -0.5
    qkv_w = np.asarray(qkv_w, np.float32)
    w_qk = np.concatenate(
        [qkv_w[:, :DIM] * scale, qkv_w[:, DIM:QK]], axis=1).astype(_BF16)
    w_v = np.ascontiguousarray(qkv_w[:, QK:]).astype(_BF16)
    pw = np.asarray(proj_w, np.float32).astype(_BF16)
    bias = np.asarray(rpb_table)[:, np.asarray(rel_index)]       # [H, nq, nk]
    expB = np.exp(bias.transpose(0, 2, 1)).astype(_BF16)          # [H, nk, nq]
    expB = np.ascontiguousarray(expB)
    qkv_b = np.asarray(qkv_b, np.float32)
    qkb = np.concatenate([qkv_b[:DIM] * scale, qkv_b[DIM:QK]])
    vb = qkv_b[QK:]
    has_qkb = bool(np.any(qkb))
    has_vb = bool(np.any(vb))
    has_pb = bool(np.any(np.asarray(proj_b)))

    ind = np.zeros((H, DIM), dtype=_BF16)
    for h in range(H):
        ind[h, h * 64:(h + 1) * 64] = 1.0
    shared = {"w_qk": w_qk, "w_v": w_v, "proj_w": pw, "expB": expB, "ind": ind}
    if has_qkb:
        shared["qkb"] = np.ascontiguousarray(qkb.reshape(12, 128)).astype(np.float32)
    if has_vb:
        shared["vb"] = vb.reshape(1, DIM).astype(_BF16)
    if has_pb:
        shared["pb"] = np.asarray(proj_b).reshape(1, DIM).astype(_BF16)

    in_maps = []
    for c in range(N_CORES):
        xs = np.asarray(x[c * B_LOC:(c + 1) * B_LOC], np.float32)
        xT = np.ascontiguousarray(xs.reshape(B_LOC * N, DIM).T).astype(_BF16)
        m = {"xT": xT}
        m.update(shared)
        in_maps.append(m)
    return in_maps, has_qkb, has_vb, has_pb


_NC_CACHE = {}


def kernel(x, qkv_w, qkv_b, proj_w, proj_b, rpb_table, rel_index):
    from concourse.bass_utils import run_bass_kernel_spmd

    in_maps, has_qkb, has_vb, has_pb = prep_host(
        x, qkv_w, qkv_b, proj_w, proj_b, rpb_table, rel_index)
    key = (has_qkb, has_vb, has_pb)
    if key not in _NC_CACHE:
        _NC_CACHE[key] = build_nc(B_LOC, has_qkb, has_vb, has_pb)
    nc = _NC_CACHE[key]
    res = run_bass_kernel_spmd(nc, in_maps, core_ids=list(range(N_CORES)))
    out = np.concatenate(
        [res.results[c]["out"].reshape(B_LOC, N, DIM) for c in range(N_CORES)],
        axis=0)
    return out.astype(np.float32)


# revision 61
# speedup vs baseline: 1.0114x; 1.0034x over previous
"""Trainium2 Bass kernel for nn_Attention_81716047774180.

Dense transformer attention block:
  qkv = x @ qkv_w + qkv_b ; split into q,k,v heads [B,H,N,d]
  attn = softmax(q k^T * scale + rel_pos_bias) ; out = (attn @ v) @ proj_w + proj_b

Distribution: pure data-parallel over batch B=64 across 8 NeuronCores
(8 batches per core, no collectives).

Device algorithm (per core, bf16 compute, fp32 accumulation):
  - host folds: scale into q-weights, rel-index gather + exp + transpose into
    an expB table, x transposed to [DIM, tokens] so no on-device transposes.
  - qk^T part computed transposed ([outdim, token]) so q^T,k^T land in [d, N]
    layout; v computed direct ([token, outdim]) with a ones column appended
    per head (gives softmax denominators for free from the P@V matmul).
  - S^T = k q^T per head ([nk, nq]) -> exp -> * expB -> P^T, so P@V needs no
    transposes: out^T[d, nq] = v^T P^T via lhsT=v.  S^T matmuls of even/odd
    heads alternate PE row groups (tile_position auto) so they co-execute.
  - denominators of all 12 heads batched into one [12, N] reciprocal
    (via base-64 staging row + tiny partition-shift DMAs), broadcast over
    partitions with indicator-matrix matmuls, normalization fused into an
    in-place multiply on attn^T.
  - software pipeline over batches: S^T burst (PE) -> next batch's
    projections (PE) overlap the exp/mul wavefront (ACT/DVE) -> P@V ->
    normalize -> proj.
"""

import sys

import numpy as np
import ml_dtypes

if "/opt/trn_rl_repo" not in sys.path:
    sys.path.insert(0, "/opt/trn_rl_repo")

B, N, DIM, H, d = 64, 320, 768, 12, 64
N_CORES = 8
B_LOC = B // N_CORES
QK = 2 * DIM          # 1536 (q^T and k^T rows)
NKC = [128, 128, 64]  # nk chunking of N=320
MT = [128, 128, 64]   # token chunking of N=320
VW = 65               # v columns per head incl. ones column

_BF16 = ml_dtypes.bfloat16


def build_nc(n_batches=B_LOC, has_qkb=False, has_vb=False, has_pb=False):
    import concourse.bass as bass  # noqa: F401
    import concourse.tile as tile
    from concourse import bacc, mybir
    from contextlib import ExitStack

    bf16 = mybir.dt.bfloat16
    f32 = mybir.dt.float32
    ACT = mybir.ActivationFunctionType

    nc = bacc.Bacc("TRN2", target_bir_lowering=False, debug=False,
                   num_devices=N_CORES)

    toks = n_batches * N
    ind_d = nc.dram_tensor("ind", [H, DIM], bf16, kind="ExternalInput").ap()
    xT_d = nc.dram_tensor("xT", [DIM, toks], bf16, kind="ExternalInput").ap()
    wqk_d = nc.dram_tensor("w_qk", [DIM, QK], bf16, kind="ExternalInput").ap()
    wv_d = nc.dram_tensor("w_v", [DIM, DIM], bf16, kind="ExternalInput").ap()
    pw_d = nc.dram_tensor("proj_w", [DIM, DIM], bf16, kind="ExternalInput").ap()
    eB_d = nc.dram_tensor("expB", [H, N, N], bf16, kind="ExternalInput").ap()
    if has_qkb:
        qkb_d = nc.dram_tensor("qkb", [12, 128], f32, kind="ExternalInput").ap()
    if has_vb:
        vb_d = nc.dram_tensor("vb", [1, DIM], bf16, kind="ExternalInput").ap()
    if has_pb:
        pb_d = nc.dram_tensor("pb", [1, DIM], bf16, kind="ExternalInput").ap()
    out_d = nc.dram_tensor("out", [toks, DIM], f32, kind="ExternalOutput").ap()

    with tile.TileContext(nc) as tc, ExitStack() as ctx:
        sing = ctx.enter_context(tc.tile_pool(name="sing", bufs=1))
        qkT_p = ctx.enter_context(tc.tile_pool(name="qkT", bufs=2))
        v_p = ctx.enter_context(tc.tile_pool(name="v", bufs=3))
        pe_p = ctx.enter_context(tc.tile_pool(name="pe", bufs=6))
        pt_p = ctx.enter_context(tc.tile_pool(name="pt", bufs=76))
        rc_p = ctx.enter_context(tc.tile_pool(name="rc", bufs=2))
        aT_p = ctx.enter_context(tc.tile_pool(name="aT", bufs=2))
        o_p = ctx.enter_context(tc.tile_pool(name="o", bufs=2))
        # Separate PSUM pools so projection matmuls are not slot-blocked
        # behind attention tiles waiting on ACT exps (8 banks total).
        ps_s = ctx.enter_context(tc.tile_pool(name="ps_s", bufs=3, space="PSUM"))
        ps_o = ctx.enter_context(tc.tile_pool(name="ps_o", bufs=3, space="PSUM"))
        ps_g = ctx.enter_context(tc.tile_pool(name="ps_g", bufs=2, space="PSUM"))

        # ---- resident constants ----
        # DMA order matters for the ramp: xT + qkv weights feed the first
        # projections, so they go first; expB is not read until the first
        # S^T results exp (~20us in), proj weights not until later still.
        wqk_t = []
        wv_t = []
        pw_t = []
        xT_t = []
        for kc in range(6):
            t = sing.tile([128, toks], bf16, tag=f"xT{kc}", name=f"xT{kc}")
            nc.sync.dma_start(t[:], xT_d[kc * 128:(kc + 1) * 128, :])
            xT_t.append(t)
            t = sing.tile([128, QK], bf16, tag=f"wqk{kc}", name=f"wqk{kc}")
            nc.sync.dma_start(t[:], wqk_d[kc * 128:(kc + 1) * 128, :])
            wqk_t.append(t)
        for kc in range(6):
            t = sing.tile([128, DIM], bf16, tag=f"wv{kc}", name=f"wv{kc}")
            nc.sync.dma_start(t[:], wv_d[kc * 128:(kc + 1) * 128, :])
            wv_t.append(t)
        eB_t = [[None] * 3 for _ in range(H)]
        for h in range(H):
            for c in range(3):
                ck = NKC[c]
                t = sing.tile([128, N], bf16, tag=f"eB{h}_{c}",
                              name=f"eB{h}_{c}")
                nc.sync.dma_start(t[:ck, :], eB_d[h, c * 128:c * 128 + ck, :])
                eB_t[h][c] = t
        for kc in range(6):
            t = sing.tile([128, DIM], bf16, tag=f"pw{kc}", name=f"pw{kc}")
            nc.sync.dma_start(t[:], pw_d[kc * 128:(kc + 1) * 128, :])
            pw_t.append(t)
        ind_t = sing.tile([128, DIM], bf16, tag="ind")
        nc.sync.dma_start(ind_t[:H, :], ind_d[:, :])
        if has_qkb:
            qkb_t = sing.tile([128, 12], f32, tag="qkb")
            nc.sync.dma_start(qkb_t[:], qkb_d.rearrange("t p -> p t"))
        if has_vb or has_pb:
            ones_t = sing.tile([128, 128], bf16, tag="ones")
            nc.vector.memset(ones_t[:], 1.0)
        if has_vb:
            vb_t = sing.tile([1, DIM], bf16, tag="vb")
            nc.sync.dma_start(vb_t[:], vb_d[:, :])
        if has_pb:
            pb_t = sing.tile([1, DIM], bf16, tag="pb")
            nc.sync.dma_start(pb_t[:], pb_d[:, :])

        def qkv_units(b, boost=False):
            """qk^T + v projection units for batch b -> (units, state).
            boost=True (prologue only): round-robin all three PSUM pools,
            which are otherwise idle before the pipeline fills."""
            t0 = b * N
            qkT = [qkT_p.tile([128, N], bf16, tag=f"qkT{mt}", name=f"qkT{mt}")
                   for mt in range(12)]
            vt = [v_p.tile([128, H * VW], bf16, tag=f"v{mt}", name=f"v{mt}")
                  for mt in range(3)]
            pools = [ps_g, ps_s, ps_o] if boost else [ps_g]
            tags = ["psg", "pss", "pso"] if boost else ["psg"]
            pidx = [0]

            def pick():
                p, tg = pools[pidx[0] % len(pools)], tags[pidx[0] % len(tags)]
                pidx[0] += 1
                return p.tile([128, 384], f32, tag=tg, name=tg)

            def qkT_unit(mt):
                ps = pick()
                for kc in range(6):
                    nc.tensor.matmul(
                        ps[:, :N],
                        lhsT=wqk_t[kc][:, mt * 128:(mt + 1) * 128],
                        rhs=xT_t[kc][:, t0:t0 + N],
                        start=(kc == 0), stop=(kc == 5))
                if has_qkb:
                    nc.scalar.activation(qkT[mt][:], ps[:, :N], ACT.Copy,
                                         bias=qkb_t[:, mt:mt + 1])
                else:
                    nc.vector.tensor_copy(qkT[mt][:], ps[:, :N])

            def v_unit(mt, nh):
                rows = MT[mt]
                t = vt[mt]
                ps = pick()
                for kc in range(6):
                    nc.tensor.matmul(
                        ps[:rows, :384],
                        lhsT=xT_t[kc][:, t0 + mt * 128:t0 + mt * 128 + rows],
                        rhs=wv_t[kc][:, nh * 384:(nh + 1) * 384],
                        start=(kc == 0), stop=(kc == 5 and not has_vb))
                if has_vb:
                    nc.tensor.matmul(
                        ps[:rows, :384],
                        lhsT=ones_t[0:1, 0:rows],
                        rhs=vb_t[0:1, nh * 384:(nh + 1) * 384],
                        start=False, stop=True)
                nc.scalar.activation(
                    t.rearrange("p (h c) -> p h c", c=VW)[:rows, nh * 6:(nh + 1) * 6, 0:64],
                    ps.rearrange("p (h c) -> p h c", c=64)[:rows, 0:6, :],
                    ACT.Copy)
                if nh == 1:
                    nc.vector.memset(
                        t.rearrange("p (h c) -> p h c", c=VW)[:rows, :, 64:65],
                        1.0)

            units = [lambda mt=mt: qkT_unit(mt) for mt in range(12)]
            units += [lambda mt=mt, nh=nh: v_unit(mt, nh)
                      for mt in range(3) for nh in range(2)]
            return units, (qkT, vt)

        def score_units(b, state, pTs):
            """18 units, one per (pair, chunk): S^T of even+odd head (in
            different PE row groups, so they co-execute) + exp + bias-mul.
            The c==0 bias-mul runs on the otherwise idle GpSimd engine."""
            qkT, vt = state

            def unit(j, c):
                ck = NKC[c]
                for r in range(2):
                    h = 2 * j + r
                    rb = r * 64
                    ps = ps_s.tile([128, N], f32, tag="pss", name="pss")
                    nc.tensor.matmul(
                        ps[:ck, :N],
                        lhsT=qkT[6 + j][rb:rb + 64, c * 128:c * 128 + ck],
                        rhs=qkT[j][rb:rb + 64, 0:N],
                        start=True, stop=True)
                    pexp = pe_p.tile([128, N], bf16, tag="pexp", name="pexp")
                    nc.scalar.activation(pexp[:ck, :], ps[:ck, :N], ACT.Exp)
                    pT = pt_p.tile([128, N], bf16, tag="pT", name="pT")
                    eng = nc.gpsimd if c <= 1 else nc.vector
                    eng.tensor_mul(pT[:ck, :], pexp[:ck, :], eB_t[h][c][:ck, :])
                    pTs[h][c] = pT

            return [lambda j=j, c=c: unit(j, c)
                    for j in range(6) for c in range(3)]

        def av_units(b, state, pTs, attnT, dens, dstage, boost=False):
            """12 units: P@V accumulation + denom extraction + unnorm evict.
            boost=True (last batch): also borrow the idle scores PSUM pool."""
            qkT, vt = state

            def unit(h):
                j, r = divmod(h, 2)
                rb = r * 64
                if boost and h % 2 == 1:
                    po = ps_s.tile([128, N], f32, tag="pss", name="pss")
                else:
                    po = ps_o.tile([128, N], f32, tag="pso", name="pso")
                for c in range(3):
                    ck = NKC[c]
                    nc.tensor.matmul(
                        po[0:VW, :N],
                        lhsT=vt[c][0:ck, h * VW:(h + 1) * VW],
                        rhs=pTs[h][c][0:ck, :],
                        start=(c == 0), stop=(c == 2))
                # denom row 64 -> a 32-aligned staging slot (engines cannot
                # write non-32-aligned partitions); once a slot group of 4
                # heads is staged, one partition-strided DMA packs them into
                # rows 4f..4f+3 of `dens`.
                sr, sc_ = 32 * (h % 4), (h // 4) * N
                with nc.allow_low_precision(reason="softmax denom in bf16"):
                    nc.vector.tensor_copy(dstage[sr:sr + 1, sc_:sc_ + N],
                                          po[64:65, :N])
                if h % 4 == 3:
                    f = h // 4
                    nc.sync.dma_start(
                        out=dens[4 * f:4 * f + 4, :],
                        in_=dstage[0:128:32, f * N:(f + 1) * N])
                nc.scalar.activation(attnT[j][rb:rb + 64, :], po[0:64, :N],
                                     ACT.Copy)

            return [lambda h=h: unit(h) for h in range(H)]

        def norm_proj_units(b, attnT, dens, boost=False):
            """Batched reciprocal, per-pair normalize, projection + out."""
            t0 = b * N
            den_r = rc_p.tile([128, N], bf16, tag="den_r", name="den_r")

            def recip_unit():
                with nc.allow_low_precision(reason="softmax denom recip bf16"):
                    nc.vector.reciprocal(den_r[:H, :], dens[:H, :])

            def norm_unit(j):
                ps_b = ps_o.tile([128, N], f32, tag="pso", name="pso")
                nc.tensor.matmul(
                    ps_b[:, :N],
                    lhsT=ind_t[0:H, j * 128:(j + 1) * 128],
                    rhs=den_r[0:H, :],
                    start=True, stop=True)
                nc.vector.tensor_mul(attnT[j][:], attnT[j][:], ps_b[:, :N])

            def proj_unit(mt):
                rows = MT[mt]
                o_t = o_p.tile([128, DIM], f32, tag="o", name="o")
                for nh in range(2):
                    if boost and nh == 1:
                        ps = ps_s.tile([128, 384], f32, tag="pss", name="pss")
                    else:
                        ps = ps_g.tile([128, 384], f32, tag="psg", name="psg")
                    for j in range(6):
                        nc.tensor.matmul(
                            ps[:rows, :384],
                            lhsT=attnT[j][:, mt * 128:mt * 128 + rows],
                            rhs=pw_t[j][:, nh * 384:(nh + 1) * 384],
                            start=(j == 0), stop=(j == 5 and not has_pb))
                    if has_pb:
                        nc.tensor.matmul(
                            ps[:rows, :384],
                            lhsT=ones_t[0:1, 0:rows],
                            rhs=pb_t[0:1, nh * 384:(nh + 1) * 384],
                            start=False, stop=True)
                    nc.vector.tensor_copy(
                        o_t[:rows, nh * 384:(nh + 1) * 384], ps[:rows, :384])
                nc.sync.dma_start(
                    out_d[t0 + mt * 128:t0 + mt * 128 + rows, :], o_t[:rows, :])

            return ([recip_unit]
                    + [lambda j=j: norm_unit(j) for j in range(6)]
                    + [lambda mt=mt: proj_unit(mt) for mt in range(3)])

        # Three-deep software pipeline over batches. Each step interleaves:
        #   - batch b's S^T/exp/bias-mul units   (PE + ACT/DVE wavefront)
        #   - batch b+1's qkT/v projection units (dense PE, independent)
        #   - batch b-1's P@V / normalize / proj (inputs all ready -> these
        #     fill every stall the exp wavefront would otherwise cause)
        # The three streams use disjoint PSUM pools (3+3+2 banks).
        qv_units, state = qkv_units(0, boost=True)
        for u in qv_units:
            u()
        tail = []          # av/norm/proj units of batch b-1
        prev_ctx = None
        for b in range(n_batches):
            attnT = [aT_p.tile([128, N], bf16, tag=f"aT{j}", name=f"aT{j}")
                     for j in range(6)]
            dens = rc_p.tile([128, N], bf16, tag="dens", name="dens")
            dstage = rc_p.tile([128, 3 * N], bf16, tag="dstage", name="dstage")
            pTs = [[None] * 3 for _ in range(H)]
            sc = score_units(b, state, pTs)
            if b + 1 < n_batches:
                qv, nstate = qkv_units(b + 1)
            else:
                qv, nstate = [], None
            ns, nq, nt = len(sc), len(qv), len(tail)
            for i in range(max(ns, nq, nt)):
                if i < nt:
                    tail[i]()
                if i < ns:
                    sc[i]()
                if i < nq:
                    qv[i]()
            last = b == n_batches - 1
            tail = (av_units(b, state, pTs, attnT, dens, dstage, boost=last)
                    + norm_proj_units(b, attnT, dens, boost=last))
            state = nstate
        for u in tail:
            u()

    nc.compile()
    return nc


def prep_host(x, qkv_w, qkv_b, proj_w, proj_b, rpb_table, rel_index):
    """Host-side preprocessing: fold scale/gather/exp/transposes."""
    scale = d ** -0.5
    qkv_w = np.asarray(qkv_w, np.float32)
    w_qk = np.concatenate(
        [qkv_w[:, :DIM] * scale, qkv_w[:, DIM:QK]], axis=1).astype(_BF16)
    w_v = np.ascontiguousarray(qkv_w[:, QK:]).astype(_BF16)
    pw = np.asarray(proj_w, np.float32).astype(_BF16)
    bias = np.asarray(rpb_table)[:, np.asarray(rel_index)]       # [H, nq, nk]
    expB = np.exp(bias.transpose(0, 2, 1)).astype(_BF16)          # [H, nk, nq]
    expB = np.ascontiguousarray(expB)
    qkv_b = np.asarray(qkv_b, np.float32)
    qkb = np.concatenate([qkv_b[:DIM] * scale, qkv_b[DIM:QK]])
    vb = qkv_b[QK:]
    has_qkb = bool(np.any(qkb))
    has_vb = bool(np.any(vb))
    has_pb = bool(np.any(np.asarray(proj_b)))

    ind = np.zeros((H, DIM), dtype=_BF16)
    for h in range(H):
        ind[h, h * 64:(h + 1) * 64] = 1.0
    shared = {"w_qk": w_qk, "w_v": w_v, "proj_w": pw, "expB": expB, "ind": ind}
    if has_qkb:
        shared["qkb"] = np.ascontiguousarray(qkb.reshape(12, 128)).astype(np.float32)
    if has_vb:
        shared["vb"] = vb.reshape(1, DIM).astype(_BF16)
    if has_pb:
        shared["pb"] = np.asarray(proj_b).reshape(1, DIM).astype(_BF16)

    in_maps = []
    for c in range(N_CORES):
        xs = np.asarray(x[c * B_LOC:(c + 1) * B_LOC], np.float32)
        xT = np.ascontiguousarray(xs.reshape(B_LOC * N, DIM).T).astype(_BF16)
        m = {"xT": xT}
        m.update(shared)
        in_maps.append(m)
    return in_maps, has_qkb, has_vb, has_pb


_NC_CACHE = {}


def kernel(x, qkv_w, qkv_b, proj_w, proj_b, rpb_table, rel_index):
    from concourse.bass_utils import run_bass_kernel_spmd

    in_maps, has_qkb, has_vb, has_pb = prep_host(
        x, qkv_w, qkv_b, proj_w, proj_b, rpb_table, rel_index)
    key = (has_qkb, has_vb, has_pb)
    if key not in _NC_CACHE:
        _NC_CACHE[key] = build_nc(B_LOC, has_qkb, has_vb, has_pb)
    nc = _NC_CACHE[key]
    res = run_bass_kernel_spmd(nc, in_maps, core_ids=list(range(N_CORES)))
    out = np.concatenate(
        [res.results[c]["out"].reshape(B_LOC, N, DIM) for c in range(N_CORES)],
        axis=0)
    return out.astype(np.float32)
